# revision 7
# baseline (speedup 1.0000x reference)
"""MoE gate (noisy top-2 routing) on 8 Trainium2 NeuronCores.

Data-parallel: tokens sharded 8 ways, [E, D] gate weights replicated.

Per-core pipeline (2048 tokens, D=4096, E=16):
  - SWDGE DMA x in 4 MiB loads ([128, 2, 4096] = 256 tokens) casting
    fp32 -> float32r on the way in
  - PE transpose (float32r, 1.5 cyc/row) 128x128 blocks -> PSUM ->
    DVE copy to SBUF [128 d, chunk, 256 tok]
  - float32r matmul, stationary = combined (gate|noise) W^T chunk
    [128,32], moving = x^T [128, 256] -> PSUM logits^T [32, 256]
    accumulated over 32 d-chunks (1 cycle/row at N=256)
  - PE transpose back to [128 tok, 32], epilogue per 128-token tile:
    softplus via Exp+Ln (ACT), noisy logits (GpSimd), softmax (ACT Exp
    with fused accumulate + DVE reciprocal), top-3 via DVE
    max8/max_index
  - blocks are software-pipelined: block k+1's load+transpose stage is
    emitted before block k's matmul+epilogue stage so the PE never
    drains below the HAM activity window.
  - stats (load-balancing loss, counts, entropy, ...) from the full
    gathered gate_weights / indices on host; tokens whose top-3 margin
    is within the float32r error band are re-ranked exactly on host.
"""

import sys

if "/opt/trn_rl_repo" not in sys.path:
    sys.path.insert(0, "/opt/trn_rl_repo")

import numpy as np

B, S, D, E = 4, 4096, 4096, 16
NCORES = 8
N = B * S
NTOK = N // NCORES          # 2048 tokens per core
NTILES = NTOK // 128        # 16 tiles of 128 tokens
BLKTOK = 256                # tokens per pipelined block (one 4 MiB DMA)
NBLK = NTOK // BLKTOK       # 8 blocks
NCHUNK = D // 128           # 32 d-chunks
TOP_K = 2
NOUT = 3                    # top-3 exported for host margin check
CAPACITY_FACTOR = 1.25
NOISE_EPS = 0.01
MARGIN = 3e-3               # host re-rank threshold on gate-weight gaps

_CACHE = {}


def _build():
    import concourse.bass as bass
    from concourse import bacc, mybir
    from concourse.tile import TileContext
    from concourse.masks import make_identity

    F32 = mybir.dt.float32
    F32R = mybir.dt.float32r
    U32 = mybir.dt.uint32
    Exp = mybir.ActivationFunctionType.Exp
    Ln = mybir.ActivationFunctionType.Ln
    Copy = mybir.ActivationFunctionType.Copy

    nc = bacc.Bacc("TRN2", target_bir_lowering=False)
    x_in = nc.declare_dram_parameter("x", [NTOK, D], F32, isOutput=False)
    wt_in = nc.declare_dram_parameter("wt", [128, 2 * E * NCHUNK], F32, isOutput=False)
    nz_in = nc.declare_dram_parameter("nz", [128, E * NTILES], F32, isOutput=False)
    gw_out = nc.declare_dram_parameter("gw", [128, E * NTILES], F32, isOutput=True)
    tw_out = nc.declare_dram_parameter("tw", [128, NOUT * NTILES], F32, isOutput=True)
    ix_out = nc.declare_dram_parameter("ix", [128, NOUT * NTILES], U32, isOutput=True)

    with TileContext(nc) as tc:
        with tc.tile_pool(name="const", bufs=1) as const, \
             tc.tile_pool(name="xnat", bufs=2) as xnat, \
             tc.tile_pool(name="xtp", bufs=2) as xtp, \
             tc.tile_pool(name="small", bufs=4) as small, \
             tc.tile_pool(name="outs", bufs=1) as outs, \
             tc.tile_pool(name="pstr", bufs=2, space="PSUM") as pstr, \
             tc.tile_pool(name="psmm", bufs=2, space="PSUM") as psmm, \
             tc.tile_pool(name="psbt", bufs=2, space="PSUM") as psbt:

            ident = const.tile([128, 128], F32)
            make_identity(nc, ident)
            identr = const.tile([128, 128], F32R)
            nc.vector.tensor_copy(identr, ident)

            wtr = const.tile([128, 2 * E * NCHUNK], F32R)
            nc.gpsimd.dma_start(out=wtr, in_=wt_in[:, :])

            nz_s = const.tile([128, E * NTILES], F32)
            nc.sync.dma_start(out=nz_s, in_=nz_in[:, :])

            gw_st = outs.tile([128, E * NTILES], F32)
            tw_st = outs.tile([128, NOUT * NTILES], F32)
            ix_st = outs.tile([128, NOUT * NTILES], U32)

            def load_transpose(blk):
                """DMA 256 tokens (one 4 MiB cast-load), PE-transpose."""
                xt = xtp.tile([128, NCHUNK, BLKTOK], F32R)
                xn = xnat.tile([128, 2, D], F32R)
                t0 = BLKTOK * blk
                nc.gpsimd.dma_start(
                    out=xn,
                    in_=x_in[t0:t0 + BLKTOK, :].rearrange(
                        "(j p) d -> p j d", p=128
                    ),
                )
                for j in range(2):
                    toff = 128 * j
                    for g in range(4):
                        tp = pstr.tile([128, 8, 128], F32R)
                        for q in range(8):
                            c = 8 * g + q
                            nc.tensor.transpose(
                                tp[:, q, :], xn[:, j, 128 * c:128 * (c + 1)],
                                identr,
                            )
                        nc.vector.tensor_copy(
                            xt[:, 8 * g:8 * (g + 1), toff:toff + 128], tp
                        )
                return xt

            def mm_epilogue(blk, xt):
                pm = psmm.tile([32, BLKTOK], F32)
                for c in range(NCHUNK):
                    nc.tensor.matmul(
                        pm,
                        wtr[:, 32 * c:32 * (c + 1)],
                        xt[:, c, :],
                        start=(c == 0),
                        stop=(c == NCHUNK - 1),
                    )
                lgs = small.tile([32, BLKTOK], F32)
                nc.scalar.activation(out=lgs, in_=pm, func=Copy)

                for j in range(2):
                    ti = 2 * blk + j
                    pt = psbt.tile([128, 32], F32)
                    nc.tensor.transpose(
                        pt, lgs[:, 128 * j:128 * (j + 1)], ident[0:32, 0:32]
                    )
                    gl = pt[:, 0:E]
                    nl = pt[:, E:2 * E]
                    gsl = gw_st[:, E * ti:E * (ti + 1)]

                    sp1 = small.tile([128, E], F32)
                    nc.scalar.activation(out=sp1, in_=nl, func=Exp)
                    sp2 = small.tile([128, E], F32)
                    nc.scalar.activation(out=sp2, in_=sp1, func=Ln, bias=1.0)
                    glc = small.tile([128, E], F32)
                    nc.scalar.activation(out=glc, in_=gl, func=Copy)
                    nm = small.tile([128, E], F32)
                    nc.gpsimd.tensor_mul(nm, sp2, nz_s[:, E * ti:E * (ti + 1)])
                    lg = small.tile([128, E], F32)
                    nc.gpsimd.tensor_add(lg, glc, nm)
                    ex = small.tile([128, E], F32)
                    sume = small.tile([128, 1], F32)
                    nc.scalar.activation(out=ex, in_=lg, func=Exp, accum_out=sume)
                    rec = small.tile([128, 1], F32)
                    nc.vector.reciprocal(rec, sume)
                    nc.vector.tensor_scalar_mul(gsl, ex, rec)
                    m8 = small.tile([128, 8], F32)
                    nc.vector.max(out=m8, in_=gsl)
                    i8 = small.tile([128, 8], U32)
                    nc.vector.max_index(out=i8, in_max=m8, in_values=gsl)
                    nc.gpsimd.tensor_copy(
                        ix_st[:, NOUT * ti:NOUT * (ti + 1)], i8[:, 0:NOUT]
                    )
                    s2 = small.tile([128, 1], F32)
                    nc.gpsimd.tensor_add(s2, m8[:, 0:1], m8[:, 1:2])
                    nc.gpsimd.tensor_scalar_add(s2, s2, 1e-8)
                    rec2 = small.tile([128, 1], F32)
                    nc.vector.reciprocal(rec2, s2)
                    nc.vector.tensor_scalar_mul(
                        tw_st[:, NOUT * ti:NOUT * (ti + 1)], m8[:, 0:NOUT], rec2
                    )

            prev = None
            for blk in range(NBLK):
                xt = load_transpose(blk)
                if prev is not None:
                    mm_epilogue(blk - 1, prev)
                prev = xt
            mm_epilogue(NBLK - 1, prev)

            nc.sync.dma_start(out=gw_out[:, :], in_=gw_st)
            nc.sync.dma_start(out=tw_out[:, :], in_=tw_st)
            nc.sync.dma_start(out=ix_out[:, :], in_=ix_st)

    nc.compile()
    return nc


def _get_nc():
    if "nc" not in _CACHE:
        _CACHE["nc"] = _build()
    return _CACHE["nc"]


def _prep_inputs(x, W_gate, W_noise, noise):
    xf = np.ascontiguousarray(x.reshape(N, D), dtype=np.float32)
    nzf = (noise.reshape(N, E) * np.float32(NOISE_EPS)).astype(np.float32)

    # combined stationary W^T, chunk-major: wt[p, 32*c + j] = Wc[j, 128*c + p]
    Wc = np.concatenate([W_gate, W_noise], axis=0).astype(np.float32)  # [32, D]
    wt = np.ascontiguousarray(
        Wc.reshape(2 * E, NCHUNK, 128).transpose(2, 1, 0).reshape(128, NCHUNK * 2 * E)
    )

    in_maps = []
    for ci in range(NCORES):
        xs = xf[ci * NTOK:(ci + 1) * NTOK]
        nzs = nzf[ci * NTOK:(ci + 1) * NTOK]
        # p-major noise staging: nz[p, 16*i + e] = nzs[128*i + p, e]
        nzp = np.ascontiguousarray(
            nzs.reshape(NTILES, 128, E).transpose(1, 0, 2).reshape(128, NTILES * E)
        )
        in_maps.append({"x": np.ascontiguousarray(xs), "wt": wt, "nz": nzp})
    return in_maps


def _unshard(results):
    gw_parts, tw_parts, ix_parts = [], [], []
    for r in results:
        gw_parts.append(
            r["gw"].reshape(128, NTILES, E).transpose(1, 0, 2).reshape(NTOK, E)
        )
        tw_parts.append(
            r["tw"].reshape(128, NTILES, NOUT).transpose(1, 0, 2).reshape(NTOK, NOUT)
        )
        ix_parts.append(
            r["ix"].reshape(128, NTILES, NOUT).transpose(1, 0, 2).reshape(NTOK, NOUT)
        )
    gw = np.concatenate(gw_parts, axis=0)
    tw3 = np.concatenate(tw_parts, axis=0)
    ix3 = np.concatenate(ix_parts, axis=0).astype(np.int64)
    return gw, ix3, tw3


def _host_rerank(x, W_gate, W_noise, noise, gw, ix3, tw3):
    """Re-rank tokens whose top-3 gate-weight margins are inside the
    float32r error band, using an exact float64 recompute."""
    rows = np.arange(N)
    g0 = gw[rows, ix3[:, 0]]
    g1 = gw[rows, ix3[:, 1]]
    g2 = gw[rows, ix3[:, 2]]
    risky = ((g0 - g1) < MARGIN) | ((g1 - g2) < MARGIN)
    idx = ix3[:, :TOP_K].astype(np.int32)
    tw = np.ascontiguousarray(tw3[:, :TOP_K])
    if risky.any():
        sel = np.where(risky)[0]
        xf = x.reshape(N, D)
        xs = xf[sel].astype(np.float64)
        gl = xs @ W_gate.astype(np.float64).T
        nl = xs @ W_noise.astype(np.float64).T
        lg = gl + noise.reshape(N, E)[sel].astype(np.float64) * np.log1p(
            np.exp(nl)
        ) * NOISE_EPS
        p = np.exp(lg - lg.max(axis=1, keepdims=True))
        p /= p.sum(axis=1, keepdims=True)
        order = np.argsort(-p, axis=1, kind="stable")[:, :TOP_K]
        idx[sel] = order.astype(np.int32)
        w = np.take_along_axis(p, order, axis=1)
        wn = w / (w.sum(axis=1, keepdims=True) + 1e-8)
        tw[sel] = wn.astype(np.float32)
    return idx, tw


def kernel(x, W_gate, W_noise, noise):
    from concourse.bass_utils import run_bass_kernel_spmd

    nc = _get_nc()
    in_maps = _prep_inputs(x, W_gate, W_noise, noise)
    res = run_bass_kernel_spmd(nc, in_maps, list(range(NCORES)))
    gw, ix3, tw3 = _unshard(res.results)
    ix, tw = _host_rerank(x, W_gate, W_noise, noise, gw, ix3, tw3)

    # host-side scalar stats from the full gathered outputs
    expert_usage = gw.mean(axis=0, dtype=np.float32)
    load_balancing_loss = np.float32(
        ((expert_usage - np.float32(1.0 / E)) ** 2).mean(dtype=np.float32)
    )
    expert_counts = np.bincount(ix.reshape(-1), minlength=E).astype(np.float32)
    capacity_per_expert = np.float32(N * CAPACITY_FACTOR / E)
    capacity_utilization = expert_counts / capacity_per_expert
    cap_mean = np.float32(capacity_utilization.mean(dtype=np.float32))
    cap_std = np.float32(capacity_utilization.std(ddof=1))
    probs = expert_counts / expert_counts.sum() + np.float32(1e-8)
    entropy = np.float32(-(probs * np.log(probs)).sum())
    overflow_rate = np.float32(
        (capacity_utilization > 1.0).astype(np.float32).mean(dtype=np.float32)
    )

    return (
        gw.reshape(B, S, E),
        ix.reshape(B, S, TOP_K),
        tw.reshape(B, S, TOP_K),
        load_balancing_loss,
        cap_mean,
        cap_std,
        entropy,
        overflow_rate,
    )


# revision 8
# speedup vs baseline: 1.0968x; 1.0968x over previous
"""MoE gate (noisy top-2 routing) on 8 Trainium2 NeuronCores.

Data-parallel: tokens sharded 8 ways, [E, D] gate weights replicated.

Per-core pipeline (2048 tokens, D=4096, E=16):
  - SWDGE DMA x in 4 MiB loads ([128, 2, 4096] = 256 tokens) casting
    fp32 -> float32r on the way in
  - PE transpose (float32r, 1.5 cyc/row) 128x128 blocks -> PSUM ->
    DVE copy to SBUF [128 d, chunk, 256 tok]
  - float32r matmul, stationary = combined (gate|noise) W^T chunk
    [128,32], moving = x^T [128, 256] -> PSUM logits^T [32, 256]
    accumulated over 32 d-chunks (1 cycle/row at N=256)
  - PE transpose back to [128 tok, 32], epilogue per 128-token tile:
    softplus via Exp+Ln (ACT), noisy logits (GpSimd), softmax (ACT Exp
    with fused accumulate + DVE reciprocal), top-3 via DVE
    max8/max_index
  - blocks are software-pipelined: block k+1's load+transpose stage is
    emitted before block k's matmul+epilogue stage so the PE never
    drains below the HAM activity window.
  - stats (load-balancing loss, counts, entropy, ...) from the full
    gathered gate_weights / indices on host; tokens whose top-3 margin
    is within the float32r error band are re-ranked exactly on host.
"""

import sys

if "/opt/trn_rl_repo" not in sys.path:
    sys.path.insert(0, "/opt/trn_rl_repo")

import numpy as np

B, S, D, E = 4, 4096, 4096, 16
NCORES = 8
N = B * S
NTOK = N // NCORES          # 2048 tokens per core
NTILES = NTOK // 128        # 16 tiles of 128 tokens
BLKTOK = 256                # tokens per pipelined block (one 4 MiB DMA)
NBLK = NTOK // BLKTOK       # 8 blocks
NCHUNK = D // 128           # 32 d-chunks
TOP_K = 2
NOUT = 3                    # top-3 exported for host margin check
CAPACITY_FACTOR = 1.25
NOISE_EPS = 0.01
MARGIN = 3e-3               # host re-rank threshold on gate-weight gaps

_CACHE = {}


def _build():
    import concourse.bass as bass
    from concourse import bacc, mybir
    from concourse.tile import TileContext
    from concourse.masks import make_identity

    F32 = mybir.dt.float32
    F32R = mybir.dt.float32r
    U32 = mybir.dt.uint32
    Exp = mybir.ActivationFunctionType.Exp
    Ln = mybir.ActivationFunctionType.Ln
    Copy = mybir.ActivationFunctionType.Copy

    nc = bacc.Bacc("TRN2", target_bir_lowering=False)
    x_in = nc.declare_dram_parameter("x", [NTOK, D], F32, isOutput=False)
    wt_in = nc.declare_dram_parameter("wt", [128, 2 * E * NCHUNK], F32, isOutput=False)
    nz_in = nc.declare_dram_parameter("nz", [128, E * NTILES], F32, isOutput=False)
    gw_out = nc.declare_dram_parameter("gw", [128, E * NTILES], F32, isOutput=True)
    tw_out = nc.declare_dram_parameter("tw", [128, NOUT * NTILES], F32, isOutput=True)
    ix_out = nc.declare_dram_parameter("ix", [128, NOUT * NTILES], U32, isOutput=True)

    with TileContext(nc) as tc:
        with tc.tile_pool(name="const", bufs=1) as const, \
             tc.tile_pool(name="xnat", bufs=2) as xnat, \
             tc.tile_pool(name="xtp", bufs=2) as xtp, \
             tc.tile_pool(name="small", bufs=4) as small, \
             tc.tile_pool(name="outs", bufs=1) as outs, \
             tc.tile_pool(name="pstr", bufs=2, space="PSUM") as pstr, \
             tc.tile_pool(name="psmm", bufs=2, space="PSUM") as psmm, \
             tc.tile_pool(name="psbt", bufs=2, space="PSUM") as psbt:

            ident = const.tile([128, 128], F32)
            make_identity(nc, ident)
            identr = const.tile([128, 128], F32R)
            nc.vector.tensor_copy(identr, ident)

            wtr = const.tile([128, 2 * E * NCHUNK], F32R)
            nc.gpsimd.dma_start(out=wtr, in_=wt_in[:, :])

            nz_s = const.tile([128, E * NTILES], F32)
            nc.sync.dma_start(out=nz_s, in_=nz_in[:, :])

            gw_st = outs.tile([128, E * NTILES], F32)
            tw_st = outs.tile([128, NOUT * NTILES], F32)
            ix_st = outs.tile([128, NOUT * NTILES], U32)

            def load_transpose(blk):
                """DMA 256 tokens (one 4 MiB load), PE-transpose into xt."""
                xt = xtp.tile([128, NCHUNK, BLKTOK], F32R)
                xn = xnat.tile([128, 2, D], F32)
                t0 = BLKTOK * blk
                nc.sync.dma_start(
                    out=xn,
                    in_=x_in[t0:t0 + BLKTOK, :].rearrange(
                        "(j p) d -> p j d", p=128
                    ),
                )
                for j in range(2):
                    toff = 128 * j
                    for g in range(4):
                        tp = pstr.tile([128, 8, 128], F32)
                        for q in range(8):
                            c = 8 * g + q
                            nc.tensor.transpose(
                                tp[:, q, :], xn[:, j, 128 * c:128 * (c + 1)],
                                ident,
                            )
                        nc.vector.tensor_copy(
                            xt[:, 8 * g:8 * (g + 1), toff:toff + 128], tp
                        )
                return xt

            def mm_epilogue(blk, xt):
                pm = psmm.tile([32, BLKTOK], F32)
                for c in range(NCHUNK):
                    nc.tensor.matmul(
                        pm,
                        wtr[:, 32 * c:32 * (c + 1)],
                        xt[:, c, :],
                        start=(c == 0),
                        stop=(c == NCHUNK - 1),
                    )
                lgs = small.tile([32, BLKTOK], F32)
                nc.scalar.activation(out=lgs, in_=pm, func=Copy)

                for j in range(2):
                    ti = 2 * blk + j
                    pt = psbt.tile([128, 32], F32)
                    nc.tensor.transpose(
                        pt, lgs[:, 128 * j:128 * (j + 1)], ident[0:32, 0:32]
                    )
                    gl = pt[:, 0:E]
                    nl = pt[:, E:2 * E]
                    gsl = gw_st[:, E * ti:E * (ti + 1)]

                    sp1 = small.tile([128, E], F32)
                    nc.scalar.activation(out=sp1, in_=nl, func=Exp)
                    sp2 = small.tile([128, E], F32)
                    nc.scalar.activation(out=sp2, in_=sp1, func=Ln, bias=1.0)
                    glc = small.tile([128, E], F32)
                    nc.scalar.activation(out=glc, in_=gl, func=Copy)
                    nm = small.tile([128, E], F32)
                    nc.gpsimd.tensor_mul(nm, sp2, nz_s[:, E * ti:E * (ti + 1)])
                    lg = small.tile([128, E], F32)
                    nc.gpsimd.tensor_add(lg, glc, nm)
                    ex = small.tile([128, E], F32)
                    sume = small.tile([128, 1], F32)
                    nc.scalar.activation(out=ex, in_=lg, func=Exp, accum_out=sume)
                    rec = small.tile([128, 1], F32)
                    nc.vector.reciprocal(rec, sume)
                    nc.vector.tensor_scalar_mul(gsl, ex, rec)
                    m8 = small.tile([128, 8], F32)
                    nc.vector.max(out=m8, in_=gsl)
                    i8 = small.tile([128, 8], U32)
                    nc.vector.max_index(out=i8, in_max=m8, in_values=gsl)
                    nc.gpsimd.tensor_copy(
                        ix_st[:, NOUT * ti:NOUT * (ti + 1)], i8[:, 0:NOUT]
                    )
                    s2 = small.tile([128, 1], F32)
                    nc.gpsimd.tensor_add(s2, m8[:, 0:1], m8[:, 1:2])
                    nc.gpsimd.tensor_scalar_add(s2, s2, 1e-8)
                    rec2 = small.tile([128, 1], F32)
                    nc.vector.reciprocal(rec2, s2)
                    nc.vector.tensor_scalar_mul(
                        tw_st[:, NOUT * ti:NOUT * (ti + 1)], m8[:, 0:NOUT], rec2
                    )

            prev = None
            for blk in range(NBLK):
                xt = load_transpose(blk)
                if prev is not None:
                    mm_epilogue(blk - 1, prev)
                prev = xt
            mm_epilogue(NBLK - 1, prev)

            nc.sync.dma_start(out=gw_out[:, :], in_=gw_st)
            nc.sync.dma_start(out=tw_out[:, :], in_=tw_st)
            nc.sync.dma_start(out=ix_out[:, :], in_=ix_st)

    nc.compile()
    return nc


def _get_nc():
    if "nc" not in _CACHE:
        _CACHE["nc"] = _build()
    return _CACHE["nc"]


def _prep_inputs(x, W_gate, W_noise, noise):
    xf = np.ascontiguousarray(x.reshape(N, D), dtype=np.float32)
    nzf = (noise.reshape(N, E) * np.float32(NOISE_EPS)).astype(np.float32)

    # combined stationary W^T, chunk-major: wt[p, 32*c + j] = Wc[j, 128*c + p]
    Wc = np.concatenate([W_gate, W_noise], axis=0).astype(np.float32)  # [32, D]
    wt = np.ascontiguousarray(
        Wc.reshape(2 * E, NCHUNK, 128).transpose(2, 1, 0).reshape(128, NCHUNK * 2 * E)
    )

    in_maps = []
    for ci in range(NCORES):
        xs = xf[ci * NTOK:(ci + 1) * NTOK]
        nzs = nzf[ci * NTOK:(ci + 1) * NTOK]
        # p-major noise staging: nz[p, 16*i + e] = nzs[128*i + p, e]
        nzp = np.ascontiguousarray(
            nzs.reshape(NTILES, 128, E).transpose(1, 0, 2).reshape(128, NTILES * E)
        )
        in_maps.append({"x": np.ascontiguousarray(xs), "wt": wt, "nz": nzp})
    return in_maps


def _unshard(results):
    gw_parts, tw_parts, ix_parts = [], [], []
    for r in results:
        gw_parts.append(
            r["gw"].reshape(128, NTILES, E).transpose(1, 0, 2).reshape(NTOK, E)
        )
        tw_parts.append(
            r["tw"].reshape(128, NTILES, NOUT).transpose(1, 0, 2).reshape(NTOK, NOUT)
        )
        ix_parts.append(
            r["ix"].reshape(128, NTILES, NOUT).transpose(1, 0, 2).reshape(NTOK, NOUT)
        )
    gw = np.concatenate(gw_parts, axis=0)
    tw3 = np.concatenate(tw_parts, axis=0)
    ix3 = np.concatenate(ix_parts, axis=0).astype(np.int64)
    return gw, ix3, tw3


def _host_rerank(x, W_gate, W_noise, noise, gw, ix3, tw3):
    """Re-rank tokens whose top-3 gate-weight margins are inside the
    float32r error band, using an exact float64 recompute."""
    rows = np.arange(N)
    g0 = gw[rows, ix3[:, 0]]
    g1 = gw[rows, ix3[:, 1]]
    g2 = gw[rows, ix3[:, 2]]
    risky = ((g0 - g1) < MARGIN) | ((g1 - g2) < MARGIN)
    idx = ix3[:, :TOP_K].astype(np.int32)
    tw = np.ascontiguousarray(tw3[:, :TOP_K])
    if risky.any():
        sel = np.where(risky)[0]
        xf = x.reshape(N, D)
        xs = xf[sel].astype(np.float64)
        gl = xs @ W_gate.astype(np.float64).T
        nl = xs @ W_noise.astype(np.float64).T
        lg = gl + noise.reshape(N, E)[sel].astype(np.float64) * np.log1p(
            np.exp(nl)
        ) * NOISE_EPS
        p = np.exp(lg - lg.max(axis=1, keepdims=True))
        p /= p.sum(axis=1, keepdims=True)
        order = np.argsort(-p, axis=1, kind="stable")[:, :TOP_K]
        idx[sel] = order.astype(np.int32)
        w = np.take_along_axis(p, order, axis=1)
        wn = w / (w.sum(axis=1, keepdims=True) + 1e-8)
        tw[sel] = wn.astype(np.float32)
    return idx, tw


def kernel(x, W_gate, W_noise, noise):
    from concourse.bass_utils import run_bass_kernel_spmd

    nc = _get_nc()
    in_maps = _prep_inputs(x, W_gate, W_noise, noise)
    res = run_bass_kernel_spmd(nc, in_maps, list(range(NCORES)))
    gw, ix3, tw3 = _unshard(res.results)
    ix, tw = _host_rerank(x, W_gate, W_noise, noise, gw, ix3, tw3)

    # host-side scalar stats from the full gathered outputs
    expert_usage = gw.mean(axis=0, dtype=np.float32)
    load_balancing_loss = np.float32(
        ((expert_usage - np.float32(1.0 / E)) ** 2).mean(dtype=np.float32)
    )
    expert_counts = np.bincount(ix.reshape(-1), minlength=E).astype(np.float32)
    capacity_per_expert = np.float32(N * CAPACITY_FACTOR / E)
    capacity_utilization = expert_counts / capacity_per_expert
    cap_mean = np.float32(capacity_utilization.mean(dtype=np.float32))
    cap_std = np.float32(capacity_utilization.std(ddof=1))
    probs = expert_counts / expert_counts.sum() + np.float32(1e-8)
    entropy = np.float32(-(probs * np.log(probs)).sum())
    overflow_rate = np.float32(
        (capacity_utilization > 1.0).astype(np.float32).mean(dtype=np.float32)
    )

    return (
        gw.reshape(B, S, E),
        ix.reshape(B, S, TOP_K),
        tw.reshape(B, S, TOP_K),
        load_balancing_loss,
        cap_mean,
        cap_std,
        entropy,
        overflow_rate,
    )


# revision 12
# speedup vs baseline: 1.1324x; 1.0324x over previous
"""MoE gate (noisy top-2 routing) on 8 Trainium2 NeuronCores.

Data-parallel: tokens sharded 8 ways, [E, D] gate weights replicated.

Per-core pipeline (2048 tokens, D=4096, E=16):
  - SWDGE DMA x in 4 MiB loads ([128, 2, 4096] = 256 tokens) casting
    fp32 -> float32r on the way in
  - PE transpose (float32r, 1.5 cyc/row) 128x128 blocks -> PSUM ->
    DVE copy to SBUF [128 d, chunk, 256 tok]
  - float32r matmul, stationary = combined (gate|noise) W^T chunk
    [128,32], moving = x^T [128, 256] -> PSUM logits^T [32, 256]
    accumulated over 32 d-chunks (1 cycle/row at N=256)
  - PE transpose back to [128 tok, 32], epilogue per 128-token tile:
    softplus via Exp+Ln (ACT), noisy logits (GpSimd), softmax (ACT Exp
    with fused accumulate + DVE reciprocal), top-3 via DVE
    max8/max_index
  - blocks are software-pipelined: block k+1's load+transpose stage is
    emitted before block k's matmul+epilogue stage so the PE never
    drains below the HAM activity window.
  - stats (load-balancing loss, counts, entropy, ...) from the full
    gathered gate_weights / indices on host; tokens whose top-3 margin
    is within the float32r error band are re-ranked exactly on host.
"""

import sys

if "/opt/trn_rl_repo" not in sys.path:
    sys.path.insert(0, "/opt/trn_rl_repo")

import numpy as np

B, S, D, E = 4, 4096, 4096, 16
NCORES = 8
N = B * S
NTOK = N // NCORES          # 2048 tokens per core
NTILES = NTOK // 128        # 16 tiles of 128 tokens
BLKTOK = 256                # tokens per pipelined block (one 4 MiB DMA)
NBLK = NTOK // BLKTOK       # 8 blocks
NCHUNK = D // 128           # 32 d-chunks
TOP_K = 2
NOUT = 3                    # top-3 exported for host margin check
CAPACITY_FACTOR = 1.25
NOISE_EPS = 0.01
MARGIN = 3e-3               # host re-rank threshold on gate-weight gaps

_CACHE = {}


def _build():
    import concourse.bass as bass
    from concourse import bacc, mybir
    from concourse.tile import TileContext
    from concourse.masks import make_identity

    F32 = mybir.dt.float32
    F32R = mybir.dt.float32r
    U32 = mybir.dt.uint32
    Exp = mybir.ActivationFunctionType.Exp
    Ln = mybir.ActivationFunctionType.Ln
    Copy = mybir.ActivationFunctionType.Copy

    class _Bacc(bacc.Bacc):
        # All ACT funcs used here (Exp, Ln, Copy) live in the
        # natural_log_exp_and_others table; offering only that table to
        # the act-table placer avoids per-tile table reload thrash.
        def insert_act_table_loads(self):
            from concourse.hw_specs import get_activation_tables
            import bass_rust as _bass_rust

            has_activation = any(
                isinstance(i, mybir.InstActivation)
                for b in self.main_func.blocks
                for i in b.instructions
            )
            if not has_activation:
                return
            # act_func_set_id is the index into this list, so keep every
            # entry in order; just hide Exp/Ln from the other sets so the
            # placer settles on the combined table.
            combined = "natural_log_exp_and_others"
            hide = {Exp, Ln}
            tables = [
                (k, set(v) if k == combined else set(v) - hide)
                for k, v in get_activation_tables(self.m.arch).items()
            ]
            _bass_rust.insert_act_table_loads(self, tables)

    nc = _Bacc("TRN2", target_bir_lowering=False)
    x_in = nc.declare_dram_parameter("x", [NTOK, D], F32, isOutput=False)
    wt_in = nc.declare_dram_parameter("wt", [128, 2 * E * NCHUNK], F32, isOutput=False)
    nz_in = nc.declare_dram_parameter("nz", [128, E * NTILES], F32, isOutput=False)
    gw_out = nc.declare_dram_parameter("gw", [128, E * NTILES], F32, isOutput=True)
    tw_out = nc.declare_dram_parameter("tw", [128, NOUT * NTILES], F32, isOutput=True)
    ix_out = nc.declare_dram_parameter("ix", [128, NOUT * NTILES], U32, isOutput=True)

    with TileContext(nc) as tc:
        with tc.tile_pool(name="const", bufs=1) as const, \
             tc.tile_pool(name="xnat", bufs=2) as xnat, \
             tc.tile_pool(name="xtp", bufs=2) as xtp, \
             tc.tile_pool(name="small", bufs=4) as small, \
             tc.tile_pool(name="outs", bufs=1) as outs, \
             tc.tile_pool(name="pstr", bufs=2, space="PSUM") as pstr, \
             tc.tile_pool(name="psmm", bufs=2, space="PSUM") as psmm, \
             tc.tile_pool(name="psbt", bufs=2, space="PSUM") as psbt:

            ident = const.tile([128, 128], F32)
            make_identity(nc, ident)
            identr = const.tile([128, 128], F32R)
            nc.vector.tensor_copy(identr, ident)

            wtr = const.tile([128, 2 * E * NCHUNK], F32R)
            nc.gpsimd.dma_start(out=wtr, in_=wt_in[:, :])

            nz_s = const.tile([128, E * NTILES], F32)
            nc.sync.dma_start(out=nz_s, in_=nz_in[:, :])

            gw_st = outs.tile([128, E * NTILES], F32)
            tw_st = outs.tile([128, NOUT * NTILES], F32)
            ix_st = outs.tile([128, NOUT * NTILES], U32)

            def load_transpose(blk):
                """DMA 256 tokens (one 4 MiB load), PE-transpose into xt."""
                xt = xtp.tile([128, NCHUNK, BLKTOK], F32R)
                xn = xnat.tile([128, 2, D], F32)
                t0 = BLKTOK * blk
                nc.sync.dma_start(
                    out=xn,
                    in_=x_in[t0:t0 + BLKTOK, :].rearrange(
                        "(j p) d -> p j d", p=128
                    ),
                )
                for j in range(2):
                    toff = 128 * j
                    for g in range(4):
                        tp = pstr.tile([128, 8, 128], F32)
                        for q in range(8):
                            c = 8 * g + q
                            nc.tensor.transpose(
                                tp[:, q, :], xn[:, j, 128 * c:128 * (c + 1)],
                                ident,
                            )
                        nc.vector.tensor_copy(
                            xt[:, 8 * g:8 * (g + 1), toff:toff + 128], tp
                        )
                return xt

            def mm_epilogue(blk, xt):
                pm = psmm.tile([32, BLKTOK], F32)
                for c in range(NCHUNK):
                    nc.tensor.matmul(
                        pm,
                        wtr[:, 32 * c:32 * (c + 1)],
                        xt[:, c, :],
                        start=(c == 0),
                        stop=(c == NCHUNK - 1),
                    )
                lgs = small.tile([32, BLKTOK], F32)
                nc.scalar.activation(out=lgs, in_=pm, func=Copy)

                for j in range(2):
                    ti = 2 * blk + j
                    pt = psbt.tile([128, 32], F32)
                    nc.tensor.transpose(
                        pt, lgs[:, 128 * j:128 * (j + 1)], ident[0:32, 0:32]
                    )
                    gl = pt[:, 0:E]
                    nl = pt[:, E:2 * E]
                    gsl = gw_st[:, E * ti:E * (ti + 1)]

                    sp1 = small.tile([128, E], F32)
                    nc.scalar.activation(out=sp1, in_=nl, func=Exp)
                    sp2 = small.tile([128, E], F32)
                    nc.scalar.activation(out=sp2, in_=sp1, func=Ln, bias=1.0)
                    glc = small.tile([128, E], F32)
                    nc.scalar.activation(out=glc, in_=gl, func=Copy)
                    nm = small.tile([128, E], F32)
                    nc.gpsimd.tensor_mul(nm, sp2, nz_s[:, E * ti:E * (ti + 1)])
                    lg = small.tile([128, E], F32)
                    nc.gpsimd.tensor_add(lg, glc, nm)
                    ex = small.tile([128, E], F32)
                    sume = small.tile([128, 1], F32)
                    nc.scalar.activation(out=ex, in_=lg, func=Exp, accum_out=sume)
                    rec = small.tile([128, 1], F32)
                    nc.vector.reciprocal(rec, sume)
                    nc.vector.tensor_scalar_mul(gsl, ex, rec)
                    m8 = small.tile([128, 8], F32)
                    nc.vector.max(out=m8, in_=gsl)
                    i8 = small.tile([128, 8], U32)
                    nc.vector.max_index(out=i8, in_max=m8, in_values=gsl)
                    nc.gpsimd.tensor_copy(
                        ix_st[:, NOUT * ti:NOUT * (ti + 1)], i8[:, 0:NOUT]
                    )
                    s2 = small.tile([128, 1], F32)
                    nc.gpsimd.tensor_add(s2, m8[:, 0:1], m8[:, 1:2])
                    nc.gpsimd.tensor_scalar_add(s2, s2, 1e-8)
                    rec2 = small.tile([128, 1], F32)
                    nc.vector.reciprocal(rec2, s2)
                    nc.vector.tensor_scalar_mul(
                        tw_st[:, NOUT * ti:NOUT * (ti + 1)], m8[:, 0:NOUT], rec2
                    )

            prev = None
            for blk in range(NBLK):
                xt = load_transpose(blk)
                if prev is not None:
                    mm_epilogue(blk - 1, prev)
                prev = xt
            mm_epilogue(NBLK - 1, prev)

            nc.sync.dma_start(out=gw_out[:, :], in_=gw_st)
            nc.sync.dma_start(out=tw_out[:, :], in_=tw_st)
            nc.sync.dma_start(out=ix_out[:, :], in_=ix_st)

    nc.compile()
    return nc


def _get_nc():
    if "nc" not in _CACHE:
        _CACHE["nc"] = _build()
    return _CACHE["nc"]


def _prep_inputs(x, W_gate, W_noise, noise):
    xf = np.ascontiguousarray(x.reshape(N, D), dtype=np.float32)
    nzf = (noise.reshape(N, E) * np.float32(NOISE_EPS)).astype(np.float32)

    # combined stationary W^T, chunk-major: wt[p, 32*c + j] = Wc[j, 128*c + p]
    Wc = np.concatenate([W_gate, W_noise], axis=0).astype(np.float32)  # [32, D]
    wt = np.ascontiguousarray(
        Wc.reshape(2 * E, NCHUNK, 128).transpose(2, 1, 0).reshape(128, NCHUNK * 2 * E)
    )

    in_maps = []
    for ci in range(NCORES):
        xs = xf[ci * NTOK:(ci + 1) * NTOK]
        nzs = nzf[ci * NTOK:(ci + 1) * NTOK]
        # p-major noise staging: nz[p, 16*i + e] = nzs[128*i + p, e]
        nzp = np.ascontiguousarray(
            nzs.reshape(NTILES, 128, E).transpose(1, 0, 2).reshape(128, NTILES * E)
        )
        in_maps.append({"x": np.ascontiguousarray(xs), "wt": wt, "nz": nzp})
    return in_maps


def _unshard(results):
    gw_parts, tw_parts, ix_parts = [], [], []
    for r in results:
        gw_parts.append(
            r["gw"].reshape(128, NTILES, E).transpose(1, 0, 2).reshape(NTOK, E)
        )
        tw_parts.append(
            r["tw"].reshape(128, NTILES, NOUT).transpose(1, 0, 2).reshape(NTOK, NOUT)
        )
        ix_parts.append(
            r["ix"].reshape(128, NTILES, NOUT).transpose(1, 0, 2).reshape(NTOK, NOUT)
        )
    gw = np.concatenate(gw_parts, axis=0)
    tw3 = np.concatenate(tw_parts, axis=0)
    ix3 = np.concatenate(ix_parts, axis=0).astype(np.int64)
    return gw, ix3, tw3


def _host_rerank(x, W_gate, W_noise, noise, gw, ix3, tw3):
    """Re-rank tokens whose top-3 gate-weight margins are inside the
    float32r error band, using an exact float64 recompute."""
    rows = np.arange(N)
    g0 = gw[rows, ix3[:, 0]]
    g1 = gw[rows, ix3[:, 1]]
    g2 = gw[rows, ix3[:, 2]]
    risky = ((g0 - g1) < MARGIN) | ((g1 - g2) < MARGIN)
    idx = ix3[:, :TOP_K].astype(np.int32)
    tw = np.ascontiguousarray(tw3[:, :TOP_K])
    if risky.any():
        sel = np.where(risky)[0]
        xf = x.reshape(N, D)
        xs = xf[sel].astype(np.float64)
        gl = xs @ W_gate.astype(np.float64).T
        nl = xs @ W_noise.astype(np.float64).T
        lg = gl + noise.reshape(N, E)[sel].astype(np.float64) * np.log1p(
            np.exp(nl)
        ) * NOISE_EPS
        p = np.exp(lg - lg.max(axis=1, keepdims=True))
        p /= p.sum(axis=1, keepdims=True)
        order = np.argsort(-p, axis=1, kind="stable")[:, :TOP_K]
        idx[sel] = order.astype(np.int32)
        w = np.take_along_axis(p, order, axis=1)
        wn = w / (w.sum(axis=1, keepdims=True) + 1e-8)
        tw[sel] = wn.astype(np.float32)
    return idx, tw


def kernel(x, W_gate, W_noise, noise):
    from concourse.bass_utils import run_bass_kernel_spmd

    nc = _get_nc()
    in_maps = _prep_inputs(x, W_gate, W_noise, noise)
    res = run_bass_kernel_spmd(nc, in_maps, list(range(NCORES)))
    gw, ix3, tw3 = _unshard(res.results)
    ix, tw = _host_rerank(x, W_gate, W_noise, noise, gw, ix3, tw3)

    # host-side scalar stats from the full gathered outputs
    expert_usage = gw.mean(axis=0, dtype=np.float32)
    load_balancing_loss = np.float32(
        ((expert_usage - np.float32(1.0 / E)) ** 2).mean(dtype=np.float32)
    )
    expert_counts = np.bincount(ix.reshape(-1), minlength=E).astype(np.float32)
    capacity_per_expert = np.float32(N * CAPACITY_FACTOR / E)
    capacity_utilization = expert_counts / capacity_per_expert
    cap_mean = np.float32(capacity_utilization.mean(dtype=np.float32))
    cap_std = np.float32(capacity_utilization.std(ddof=1))
    probs = expert_counts / expert_counts.sum() + np.float32(1e-8)
    entropy = np.float32(-(probs * np.log(probs)).sum())
    overflow_rate = np.float32(
        (capacity_utilization > 1.0).astype(np.float32).mean(dtype=np.float32)
    )

    return (
        gw.reshape(B, S, E),
        ix.reshape(B, S, TOP_K),
        tw.reshape(B, S, TOP_K),
        load_balancing_loss,
        cap_mean,
        cap_std,
        entropy,
        overflow_rate,
    )
